# revision 4
# baseline (speedup 1.0000x reference)
"""3-layer GCN (message passing) kernel for 8 Trainium2 NeuronCores.

Strategy (single SPMD program on 8 cores):
  - gcn(h) = (D^-1/2 (A+I) D^-1/2 h) W + b  -- weight matmul commutes past the
    normalized aggregation, so each layer is: gather+segment-sum of the table
    T = dinv*h_prev (node-major rows in HBM), then a small dense matmul on the
    core's dst shard.
  - dst nodes are sharded contiguously across the 8 cores.  Each core's edges
    are grouped into 4 "window" streams by src physical row (so gather indices
    fit int16), sorted by dst inside each stream.
  - Messages are fetched edge-major with dma_gather (128 edges per chunk);
    segment-sum is PE matmuls: stationary = message chunk [128e x 128f],
    moving = a narrow mask [128e x W] whose col j holds dinv_dst for edges of
    slot j (zero elsewhere); accumulated into PSUM where slot == local dst id.
  - A fixed schedule (D dsts per chunk per stream, mask window W, verified on
    the host with zero-row padding for slack) makes every AP offset a compile
    time constant, identical on all cores; all per-core variation is data.
  - The per-layer table for layer l+1 is rebuilt from the dst shard
    (u = dinv*relu(F^T W + b)) and exchanged with one AllGather; the rank
    concatenation of [SHARD+1, 128] blocks (data + zero row) yields exactly
    the windowed physical table layout, zero rows included.
  - Layer 3 output is mean over nodes => (c^T h2) W3 / N + b3 with
    c = per-src total edge norm; each core emits a [1,128] partial.
"""

import math
from contextlib import ExitStack

import numpy as np

import concourse.bass as bass
import concourse.bacc as bacc
import concourse.mybir as mybir
import concourse.tile as tile
from concourse.bass import _add_dep_helper

F32 = mybir.dt.float32
BF16 = mybir.dt.bfloat16
I16 = mybir.dt.int16

N_NODES = 100000
N_CORES = 8
DF = 128  # feature dim (all layers)


def default_cfg():
    import os
    bf16 = os.environ.get("GNN_BF16", "0") == "1"
    return dict(
        D=14,          # dsts per chunk per window stream
        W=24,          # mask window width (slots)
        G=int(os.environ.get("GNN_G", "8")),   # chunks per gather call
        table_bf16=bf16,
        mask_bf16=bf16,
        fallbacks=[(14, 24), (13, 24), (12, 24), (10, 32), (8, 32), (6, 48)],
    )


# ----------------------------------------------------------------------------
# Host preprocessing
# ----------------------------------------------------------------------------

def schedule_stream(dstl_sorted, shard, D, W, C_T=None):
    """Chunk schedule for one (core, window) stream.

    Returns s: int array [C+1], s[k]..s[k+1] = edge range of chunk k.
    Chunk k may only contain edges with dst < hi_k = min(D*(k+1), shard),
    capacity 128.  If C_T is None, run until all edges are emitted.
    """
    total = len(dstl_sorted)
    # eligible count per chunk limit
    s = [0]
    k = 0
    while True:
        hi = min(D * (k + 1), shard)
        e_k = np.searchsorted(dstl_sorted, hi, side="left")
        nxt = min(s[-1] + 128, int(e_k))
        s.append(nxt)
        k += 1
        if C_T is None:
            if nxt >= total and hi >= shard:
                break
            if k > 20000:
                raise RuntimeError("schedule runaway")
        else:
            if k == C_T:
                break
    s = np.asarray(s, dtype=np.int64)
    if s[-1] != total:
        raise ValueError("schedule infeasible: edges left over")
    return s


def verify_stream(dstl_sorted, s, shard, D, W):
    C = len(s) - 1
    for k in range(C):
        if s[k + 1] > s[k]:
            hi = min(D * (k + 1), shard)
            if dstl_sorted[s[k]] < hi - W:
                return False
    return True


def preprocess(edge_index, x, n_nodes, n_cores, cfg):
    """Compute all per-core data arrays + schedule constants."""
    shard = n_nodes // n_cores
    assert shard * n_cores == n_nodes
    nwin = n_cores // 2
    blk = shard + 1
    winrows = 2 * blk
    assert winrows - 1 < 2 ** 15

    src0 = edge_index[0].astype(np.int64)
    dst0 = edge_index[1].astype(np.int64)
    loop = np.arange(n_nodes, dtype=np.int64)
    src = np.concatenate([src0, loop])
    dst = np.concatenate([dst0, loop])

    deg = np.bincount(dst, minlength=n_nodes).astype(np.float32)
    dinv = (1.0 / np.sqrt(deg)).astype(np.float32)
    # c_i = sum_{e: src_e=i} dinv_i * dinv_dst_e   (incl. self loops)
    c_vec = dinv * np.bincount(
        src, weights=dinv[dst].astype(np.float64), minlength=n_nodes
    ).astype(np.float32)

    core_of = dst // shard
    p_row = src + src // shard  # physical table row
    w_of = p_row // winrows
    loc = (p_row % winrows).astype(np.int64)

    # choose (D, W) with feasibility check; also find uniform C_T
    per_core = []
    for c in range(n_cores):
        sel = np.nonzero(core_of == c)[0]
        dl = dst[sel] - c * shard
        per_core.append((sel, dl))

    G = cfg["G"]
    for (D, W) in cfg["fallbacks"]:
        try:
            streams = {}
            C_need = 0
            ok = True
            for c in range(n_cores):
                sel, dl = per_core[c]
                for w in range(nwin):
                    m = np.nonzero(w_of[sel] == w)[0]
                    es = sel[m]
                    order = np.argsort(dl[m], kind="stable")
                    es = es[order]
                    dls = dl[m][order]
                    s = schedule_stream(dls, shard, D, W)
                    if not verify_stream(dls, s, shard, D, W):
                        ok = False
                        break
                    streams[(c, w)] = (es, dls, s)
                    C_need = max(C_need, len(s) - 1)
                if not ok:
                    break
            if not ok:
                continue
            C_need = max(C_need, math.ceil(shard / D))
            C_T = math.ceil(C_need / G) * G
            # re-run all schedules at uniform C_T, re-verify
            ok = True
            for key, (es, dls, s) in list(streams.items()):
                s2 = schedule_stream(dls, shard, D, W, C_T=C_T)
                if not verify_stream(dls, s2, shard, D, W):
                    ok = False
                    break
                streams[key] = (es, dls, s2)
            if ok:
                break
        except ValueError:
            continue
    else:
        raise RuntimeError("no feasible (D, W) schedule found")

    # build idx / mask arrays
    tdt = ml_bf16() if cfg["table_bf16"] else np.float32
    mdt = ml_bf16() if cfg["mask_bf16"] else np.float32
    NT = math.ceil(shard / 128)

    idx_hbm = np.empty((n_cores, 128, nwin * C_T * 8), dtype=np.int16)
    mask_hbm = np.zeros((n_cores, 128, nwin * C_T * W), dtype=mdt)
    dinv_cols = np.zeros((n_cores, 128, NT), dtype=np.float32)
    c_cols = np.zeros((n_cores, 128, NT), dtype=np.float32)

    hi_arr = np.minimum(np.arange(1, C_T + 1) * D, shard)

    for c in range(n_cores):
        dsh = dinv[c * shard:(c + 1) * shard]
        csh = c_vec[c * shard:(c + 1) * shard]
        dc = np.zeros(NT * 128, dtype=np.float32)
        dc[:shard] = dsh
        dinv_cols[c] = dc.reshape(NT, 128).T
        cc = np.zeros(NT * 128, dtype=np.float32)
        cc[:shard] = csh
        c_cols[c] = cc.reshape(NT, 128).T
        for w in range(nwin):
            es, dls, s = streams[(c, w)]
            total = len(es)
            idx = np.full((C_T, 128), shard, dtype=np.int16)  # pad -> zero row
            msk = np.zeros((C_T, 128, W), dtype=np.float32)
            if total:
                counts = (s[1:] - s[:-1]).astype(np.int64)
                ks = np.repeat(np.arange(C_T), counts)
                rows = np.arange(total) - s[ks]
                idx[ks, rows] = loc[es].astype(np.int16)
                cols = dls - (hi_arr[ks] - W)
                assert (cols >= 0).all() and (cols < W).all()
                msk[ks, rows, cols] = dinv[dst[es]]
            # wrapped idx layout: [128, C_T*8]
            flat = idx.reshape(-1)
            w16 = flat.reshape(-1, 16).T  # [16, C_T*8]
            idx_hbm[c, :, w * C_T * 8:(w + 1) * C_T * 8] = np.tile(w16, (8, 1))
            mask_hbm[c, :, w * C_T * W:(w + 1) * C_T * W] = (
                msk.transpose(1, 0, 2).reshape(128, C_T * W).astype(mdt)
            )

    # layer-1 table: dinv * x in physical (rank-block) layout
    xp = (dinv[:, None] * x).astype(tdt)
    tbl1 = np.zeros((n_cores * blk, DF), dtype=tdt)
    for c in range(n_cores):
        tbl1[c * blk: c * blk + shard] = xp[c * shard:(c + 1) * shard]

    meta = dict(
        D=D, W=W, G=G, C_T=C_T, shard=shard, nwin=nwin, blk=blk,
        winrows=winrows, NT=NT, n_cores=n_cores,
        tdt=tdt, mdt=mdt,
    )
    data = dict(
        idx_hbm=idx_hbm, mask_hbm=mask_hbm, dinv_cols=dinv_cols,
        c_cols=c_cols, tbl1=tbl1, dinv=dinv, c_vec=c_vec,
    )
    return meta, data


def ml_bf16():
    import ml_dtypes
    return ml_dtypes.bfloat16


# ----------------------------------------------------------------------------
# Kernel builder
# ----------------------------------------------------------------------------

def build_kernel(meta):
    D, W, G, C_T = meta["D"], meta["W"], meta["G"], meta["C_T"]
    shard, nwin, blk = meta["shard"], meta["nwin"], meta["blk"]
    winrows, NT, n_cores = meta["winrows"], meta["NT"], meta["n_cores"]
    TDT = BF16 if meta["tdt"] != np.float32 else F32
    MDT = BF16 if meta["mdt"] != np.float32 else F32
    NG = C_T // G
    nbanks = math.ceil(shard / 512)

    nc = bacc.Bacc("TRN2", target_bir_lowering=False, debug=False,
                   num_devices=n_cores)

    tbl1_t = nc.dram_tensor("tbl1", [n_cores * blk, DF], TDT, kind="ExternalInput")
    idx_t = nc.dram_tensor("idxs", [128, nwin * C_T * 8], I16, kind="ExternalInput")
    mask_t = nc.dram_tensor("masks", [128, nwin * C_T * W], MDT, kind="ExternalInput")
    w1_t = nc.dram_tensor("w1", [DF, DF], F32, kind="ExternalInput")
    w2_t = nc.dram_tensor("w2", [DF, DF], F32, kind="ExternalInput")
    w3_t = nc.dram_tensor("w3", [DF, DF], F32, kind="ExternalInput")
    b1_t = nc.dram_tensor("b1r", [128, DF], F32, kind="ExternalInput")
    b2_t = nc.dram_tensor("b2r", [128, DF], F32, kind="ExternalInput")
    dinv_t = nc.dram_tensor("dinvc", [128, NT], F32, kind="ExternalInput")
    c_t = nc.dram_tensor("cc", [128, NT], F32, kind="ExternalInput")
    r_t = nc.dram_tensor("r_out", [1, DF], F32, kind="ExternalOutput")

    cc_in = nc.dram_tensor("cc_in", [blk, DF], TDT)
    cc_out = nc.dram_tensor("cc_out", [n_cores * blk, DF], TDT,
                            addr_space="Shared")

    with ExitStack() as ctx:
        tc = ctx.enter_context(tile.TileContext(nc))
        const_pool = ctx.enter_context(tc.tile_pool(name="const", bufs=1))
        ipool = ctx.enter_context(tc.tile_pool(name="idx", bufs=2 * nwin))
        mpool = ctx.enter_context(tc.tile_pool(name="mask", bufs=2 * nwin))
        gpools = [
            ctx.enter_context(tc.tile_pool(name=f"gath{w}", bufs=2))
            for w in range(nwin)
        ]
        apsum = ctx.enter_context(tc.tile_pool(name="apsum", bufs=3, space="PSUM"))
        dpsum = ctx.enter_context(tc.tile_pool(name="dpsum", bufs=2, space="PSUM"))
        vpool = ctx.enter_context(tc.tile_pool(name="vpsum", bufs=1, space="PSUM"))
        fpool = ctx.enter_context(tc.tile_pool(name="fsb", bufs=1))
        hpool = ctx.enter_context(tc.tile_pool(name="hsb", bufs=4))
        upool = ctx.enter_context(tc.tile_pool(name="usb", bufs=3))

        # constants
        zeros_sb = const_pool.tile([128, 512], TDT)
        nc.vector.memset(zeros_sb[:], 0.0)
        w1_sb = const_pool.tile([128, DF], F32, tag="w1")
        nc.sync.dma_start(w1_sb[:], w1_t[:, :])
        w2_sb = const_pool.tile([128, DF], F32, tag="w2")
        nc.sync.dma_start(w2_sb[:], w2_t[:, :])
        w3_sb = const_pool.tile([128, DF], F32, tag="w3")
        nc.sync.dma_start(w3_sb[:], w3_t[:, :])
        b1_sb = const_pool.tile([128, DF], F32, tag="b1")
        nc.sync.dma_start(b1_sb[:], b1_t[:, :])
        b2_sb = const_pool.tile([128, DF], F32, tag="b2")
        nc.sync.dma_start(b2_sb[:], b2_t[:, :])
        dinv_sb = const_pool.tile([128, NT], F32, tag="dinv")
        nc.sync.dma_start(dinv_sb[:], dinv_t[:, :])
        c_sb = const_pool.tile([128, NT], F32, tag="cvec")
        nc.sync.dma_start(c_sb[:], c_t[:, :])

        # zero row of cc_in
        zrow_wr = nc.sync.dma_start(cc_in[shard:shard + 1, :], zeros_sb[0:1, 0:DF])

        hi_arr = [min(D * (k + 1), shard) for k in range(C_T)]

        def emit_agg(table_t, gather_insts_out):
            """Aggregation for one layer; returns F_sb [128, NT*128] fp32."""
            F_sb = fpool.tile([128, NT * 128], F32, tag="F")
            banks = {}
            msg_tiles = [None] * nwin
            msk_tiles = [None] * nwin
            for k in range(C_T):
                hi = hi_arr[k]
                lo = max(hi - W, 0)
                g, r = divmod(k, G)
                if r == 0:
                    for w in range(nwin):
                        idx_sb = ipool.tile([128, G * 8], I16, tag="idx")
                        nc.sync.dma_start(
                            idx_sb[:],
                            idx_t[:, (w * C_T + k) * 8:(w * C_T + k) * 8 + G * 8])
                        msk_sb = mpool.tile([128, G * W], MDT, tag="mask")
                        nc.sync.dma_start(
                            msk_sb[:],
                            mask_t[:, (w * C_T + k) * W:(w * C_T + k) * W + G * W])
                        mt = gpools[w].tile([128, G, DF], TDT, tag=f"g{w}")
                        gi = nc.gpsimd.dma_gather(
                            mt[:], table_t[w * winrows:(w + 1) * winrows, :],
                            idx_sb[:], G * 128, G * 128, DF)
                        gather_insts_out.append(gi)
                        msg_tiles[w] = mt
                        msk_tiles[w] = msk_sb
                # bank init
                for b in {lo // 512, (hi - 1) // 512}:
                    if b not in banks:
                        t = apsum.tile([128, 512], F32, tag="agg")
                        nc.tensor.matmul(t[:, :], zeros_sb[:, 0:128],
                                         zeros_sb[:, 0:512], start=True,
                                         stop=False, skip_group_check=True)
                        banks[b] = t
                # matmuls
                b_lo, b_hi = lo // 512, (hi - 1) // 512
                segs = ([(lo, hi)] if b_lo == b_hi
                        else [(lo, (b_lo + 1) * 512), ((b_lo + 1) * 512, hi)])
                for w in range(nwin):
                    lhs = msg_tiles[w][:, r, :]
                    for (a, e) in segs:
                        bb = a // 512
                        pt = banks[bb]
                        mc0 = a - (hi - W)
                        mc1 = e - (hi - W)
                        nc.tensor.matmul(
                            pt[:, a - 512 * bb:e - 512 * bb], lhs,
                            msk_tiles[w][:, r * W + mc0:r * W + mc1],
                            start=False, stop=False, skip_group_check=True)
                # flush passed banks
                if k + 1 < C_T:
                    lo_next = max(min(D * (k + 2), shard) - W, 0)
                else:
                    lo_next = shard + 512
                for b in sorted(list(banks.keys())):
                    if 512 * (b + 1) <= lo_next:
                        width = min(512, shard - 512 * b)
                        nc.scalar.copy(F_sb[:, 512 * b:512 * b + width],
                                       banks[b][:, :width])
                        del banks[b]
            assert not banks
            return F_sb

        def emit_dense(layer, F_sb, wl_sb, bl_sb):
            """Dense phase; layer 1 writes cc_in, layer 2 accumulates v."""
            u_writes = []
            vps = None
            if layer == 2:
                vps = vpool.tile([128, 512], F32, tag="v")
            for t in range(NT):
                nt = min(128, shard - 128 * t)
                dp = dpsum.tile([128, 128], F32, tag="dense")
                nc.tensor.matmul(dp[0:nt, :], F_sb[:, 128 * t:128 * t + nt],
                                 wl_sb[:, :], start=True, stop=True,
                                 skip_group_check=True)
                hb = hpool.tile([128, 128], F32, tag="h1")
                nc.vector.tensor_add(hb[0:nt, :], dp[0:nt, :], bl_sb[0:nt, :])
                hb2 = hpool.tile([128, 128], F32, tag="h2")
                nc.scalar.activation(hb2[0:nt, :], hb[0:nt, :],
                                     mybir.ActivationFunctionType.Relu)
                if layer == 1:
                    ub = upool.tile([128, 128], TDT, tag="u")
                    nc.vector.tensor_scalar_mul(ub[0:nt, :], hb2[0:nt, :],
                                                dinv_sb[0:nt, t:t + 1])
                    wr = nc.sync.dma_start(cc_in[128 * t:128 * t + nt, :],
                                           ub[0:nt, :])
                    u_writes.append(wr)
                else:
                    nc.tensor.matmul(vps[:, 0:1], hb2[0:nt, :],
                                     c_sb[0:nt, t:t + 1], start=(t == 0),
                                     stop=(t == NT - 1), skip_group_check=True)
            return u_writes, vps

        # ---- layer 1
        g1 = []
        F1 = emit_agg(tbl1_t, g1)
        u_writes, _ = emit_dense(1, F1, w1_sb, b1_sb)

        # ---- allgather
        cc = nc.gpsimd.collective_compute(
            "AllGather", mybir.AluOpType.bypass,
            replica_groups=[list(range(n_cores))],
            ins=[cc_in.ap()], outs=[cc_out.ap()])
        for wr in u_writes + [zrow_wr]:
            _add_dep_helper(cc.ins, wr.ins, sync=True, reason="cc after table write")

        # ---- layer 2
        g2 = []
        F2 = emit_agg(cc_out, g2)
        for gi in g2:
            _add_dep_helper(gi.ins, cc.ins, sync=True, reason="gather after cc")
        _, vps = emit_dense(2, F2, w2_sb, b2_sb)

        # ---- layer 3 collapsed: r = (v^T W3)
        vsb = hpool.tile([128, 128], F32, tag="vsb")
        nc.scalar.copy(vsb[:, 0:1], vps[:, 0:1])
        rps = dpsum.tile([128, 128], F32, tag="dense")
        nc.tensor.matmul(rps[0:1, :], vsb[:, 0:1], w3_sb[:, :],
                         start=True, stop=True, skip_group_check=True)
        rsb = hpool.tile([128, 128], F32, tag="rsb")
        nc.scalar.copy(rsb[0:1, :], rps[0:1, :])
        nc.sync.dma_start(r_t[:, :], rsb[0:1, :])

    nc.compile()
    return nc


# ----------------------------------------------------------------------------
# numpy model of the device computation (for schedule validation)
# ----------------------------------------------------------------------------

def host_model(meta, data, weights, n_nodes):
    """Simulate exactly what the device computes, in numpy fp32."""
    D, W, G, C_T = meta["D"], meta["W"], meta["G"], meta["C_T"]
    shard, nwin, blk, winrows, NT = (meta["shard"], meta["nwin"], meta["blk"],
                                     meta["winrows"], meta["NT"])
    n_cores = meta["n_cores"]
    w1, b1, w2, b2, w3, b3 = weights
    tbl = data["tbl1"].astype(np.float32)
    hi_arr = np.minimum(np.arange(1, C_T + 1) * D, shard)

    r_total = np.zeros((1, DF), dtype=np.float32)
    tbl_next = np.zeros_like(tbl)
    for layer in (1, 2):
        parts = []
        for c in range(n_cores):
            agg = np.zeros((shard, DF), dtype=np.float32)
            for w in range(nwin):
                idx = data["idx_hbm"][c][:16, w * C_T * 8:(w + 1) * C_T * 8]
                flat = idx.T.reshape(-1)  # [C_T*128] undo wrap
                msk = data["mask_hbm"][c][:, w * C_T * W:(w + 1) * C_T * W]
                msk = msk.reshape(128, C_T, W).transpose(1, 0, 2).astype(np.float32)
                rows = tbl[w * winrows + flat]  # [C_T*128, DF]
                rows = rows.reshape(C_T, 128, DF)
                # agg[slot] += mask^T @ rows  per chunk
                for k in range(C_T):
                    hi = hi_arr[k]
                    lo = max(hi - W, 0)
                    contrib = msk[k].T @ rows[k]  # [W, DF]
                    agg[lo:hi] += contrib[lo - (hi - W):, :][: hi - lo]
            parts.append(agg)
        h_parts = []
        for c in range(n_cores):
            wl, bl = (w1, b1) if layer == 1 else (w2, b2)
            h = np.maximum(parts[c] @ wl + bl, 0.0)
            h_parts.append(h)
            if layer == 1:
                dsh = data["dinv"][c * shard:(c + 1) * shard]
                tbl_next[c * blk:c * blk + shard] = h * dsh[:, None]
        tbl = tbl_next
        if layer == 2:
            for c in range(n_cores):
                csh = data["c_vec"][c * shard:(c + 1) * shard]
                v = csh @ h_parts[c]
                r_total += (v @ w3)[None, :]
    return r_total / n_nodes + b3[None, :]


# ----------------------------------------------------------------------------
# entry point
# ----------------------------------------------------------------------------

_CACHE = {}


def _build_inputs_per_core(meta, data, w1, b1, w2, b2, w3):
    n_cores = meta["n_cores"]
    b1r = np.tile(np.asarray(b1, dtype=np.float32)[None, :], (128, 1))
    b2r = np.tile(np.asarray(b2, dtype=np.float32)[None, :], (128, 1))
    in_maps = []
    for c in range(n_cores):
        in_maps.append(dict(
            tbl1=np.ascontiguousarray(data["tbl1"]),
            idxs=np.ascontiguousarray(data["idx_hbm"][c]),
            masks=np.ascontiguousarray(data["mask_hbm"][c]),
            w1=np.asarray(w1, dtype=np.float32),
            w2=np.asarray(w2, dtype=np.float32),
            w3=np.asarray(w3, dtype=np.float32),
            b1r=b1r, b2r=b2r,
            dinvc=np.ascontiguousarray(data["dinv_cols"][c]),
            cc=np.ascontiguousarray(data["c_cols"][c]),
        ))
    return in_maps


def kernel(x, edge_index, w1, b1, w2, b2, w3, b3, _trace=False):
    from concourse.bass_utils import run_bass_kernel_spmd

    x = np.asarray(x)
    edge_index = np.asarray(edge_index)
    n_nodes = x.shape[0]
    cfg = default_cfg()
    meta, data = preprocess(edge_index, x, n_nodes, N_CORES, cfg)
    key = (n_nodes, meta["C_T"], meta["D"], meta["W"])
    if key not in _CACHE:
        _CACHE[key] = build_kernel(meta)
    nc = _CACHE[key]
    in_maps = _build_inputs_per_core(meta, data, w1, b1, w2, b2, w3)
    res = run_bass_kernel_spmd(nc, in_maps, core_ids=list(range(N_CORES)),
                               trace=_trace)
    r = np.zeros((1, DF), dtype=np.float32)
    for cr in res.results:
        r += cr["r_out"]
    out = r / n_nodes + np.asarray(b3, dtype=np.float32)[None, :]
    kernel._last_result = res
    return out.astype(np.float32)
